# revision 3
# baseline (speedup 1.0000x reference)
"""Canny filter Bass kernel for Trainium2, data-parallel over batch on 8 cores.

v2 redesign vs baseline (401 us):
- Host pre-sums the 3 channels (the whole pipeline depends only on
  s = sum_c img_c) and splits s into an 11-bit-mantissa hi part plus the
  residual lo part. Input DMA drops from 12.6 to 8.4 MB/core and the
  channel-sum matmuls disappear.
- Sobel verticals+horizontal taps fold into f32r matmuls (1 cyc/row vs 4
  for fp32). f32r rounds inputs to 11 mantissa bits; accumulating the
  hi and lo pieces in PSUM restores full fp32 accuracy (weights are
  exactly representable), which the NMS tie-sensitivity requires.
- NMS row shifts of q run as plain fp32 matmuls (exact); their missing
  boundary rows are patched with 1-row SBUF-SBUF DMAs from the neighbor
  blocks' q tiles instead of halo matmuls.
- Threshold/hysteresis chain in bf16 (exact on 0/1/2-valued data); the
  3x3 sum is vertical-T3 + 32*lo fold on the tensor engine, one PSUM
  compare emits the final edge map.
- Orientation: reciprocal_approx_fast only (51 ULP suffices; verified),
  gx biased by 1e-30 in its PSUM evacuation so rec(0) never occurs.
- sqrt(q) runs on a bf16 stash of q (0.1% error, tolerance 2e-2) after
  the last arctan via a bias-operand dependency -> one table switch.
"""

import os
from contextlib import ExitStack

import numpy as np
import ml_dtypes

import concourse.bacc as bacc
import concourse.tile as tile
from concourse import mybir
from concourse.bass_utils import run_bass_kernel_spmd

F32 = mybir.dt.float32
F32R = mybir.dt.float32r
I32 = mybir.dt.int32
U8 = mybir.dt.uint8
BF16 = mybir.dt.bfloat16
AF = mybir.ActivationFunctionType
ALU = mybir.AluOpType

H = W = 1024
NB = 8          # row blocks
P = 128         # rows per block
HALF = 512      # f32/f32r matmul max moving free dim
INV3 = float(np.float32(1.0) / np.float32(3.0))
K8PI = float(np.float32(8.0 / np.pi))
GXEPS = 1e-30
WP = W + 2


def _const_weights():
    """f32 [128, 2*128]: QU | QD (NMS row shifts, run as exact fp32)."""
    cw = np.zeros((P, 2 * P), np.float32)
    QU = cw[:, 0:P]
    QD = cw[:, P:2 * P]
    for m in range(P):
        if m > 0:
            QU[m - 1, m] = 1.0
        if m < P - 1:
            QD[m + 1, m] = 1.0
    return cw


def _const_weights_bf16():
    """bf16 [128, 11*128]:
    T3 | L32 | hT3 | VsP | VsM | VdH | VdC | hBP | hBM | hDH | hDC.

    All entries (0.5/1/32 and signs) are exactly representable in bf16.
    Halo blocks use rows 0/1 only (row 0 -> out row 0 from block row -1,
    row 1 -> out row 127 from block row 128).
    """
    cwb = np.zeros((P, 11 * P), np.float32)

    def blk(i):
        return cwb[:, i * P:(i + 1) * P]

    T3, L32, hT3, VsP, VsM, VdH, VdC, hBP, hBM, hDH, hDC = (
        blk(i) for i in range(11))
    for m in range(P):
        T3[m, m] = 1.0
        L32[m, m] = 32.0
        VsP[m, m] = 1.0
        VsM[m, m] = -1.0
        if m > 0:
            T3[m - 1, m] = 1.0
            VsP[m - 1, m] = 0.5
            VsM[m - 1, m] = -0.5
            VdC[m - 1, m] = -1.0
            VdH[m - 1, m] = -0.5
        if m < P - 1:
            T3[m + 1, m] = 1.0
            VsP[m + 1, m] = 0.5
            VsM[m + 1, m] = -0.5
            VdC[m + 1, m] = 1.0
            VdH[m + 1, m] = 0.5
    hT3[0, 0] = 1.0
    hT3[1, P - 1] = 1.0
    hBP[0, 0] = 0.5
    hBP[1, P - 1] = 0.5
    hBM[0, 0] = -0.5
    hBM[1, P - 1] = -0.5
    hDH[0, 0] = -0.5
    hDH[1, P - 1] = 0.5
    hDC[0, 0] = -1.0
    hDC[1, P - 1] = 1.0
    return cwb.astype(ml_dtypes.bfloat16)


def _emit(nc, tc, sh_in, sl_in, cw, cwb, o_gx, o_gy, o_gm, o_or, o_te):
    v = nc.vector
    sc = nc.scalar
    te_ = nc.tensor
    gp = nc.gpsimd

    ctx = ExitStack()
    cpool = ctx.enter_context(tc.tile_pool(name="cp", bufs=1))
    inp = ctx.enter_context(tc.tile_pool(name="inp", bufs=2))
    spp = ctx.enter_context(tc.tile_pool(name="spp", bufs=2))
    qpool = ctx.enter_context(tc.tile_pool(name="qp", bufs=9))
    updn = ctx.enter_context(tc.tile_pool(name="ud", bufs=1))
    g2p = ctx.enter_context(tc.tile_pool(name="g2p", bufs=1))
    sb1 = ctx.enter_context(tc.tile_pool(name="sb1", bufs=1))
    msk = ctx.enter_context(tc.tile_pool(name="msk", bufs=1))
    pip = ctx.enter_context(tc.tile_pool(name="pip", bufs=2))
    thp = ctx.enter_context(tc.tile_pool(name="thp", bufs=1))
    thp2 = ctx.enter_context(tc.tile_pool(name="thp2", bufs=2))
    outp = ctx.enter_context(tc.tile_pool(name="outp", bufs=1))
    hpool = ctx.enter_context(tc.tile_pool(name="hp", bufs=3))
    psA = ctx.enter_context(tc.tile_pool(name="psA", bufs=2, space="PSUM"))
    psU = ctx.enter_context(tc.tile_pool(name="psU", bufs=1, space="PSUM"))
    psD = ctx.enter_context(tc.tile_pool(name="psD", bufs=1, space="PSUM"))

    cwt = cpool.tile([P, 2 * P], F32, tag="cw")
    nc.sync.dma_start(cwt[:], cw[:])
    cwbt = cpool.tile([P, 11 * P], BF16, tag="cwb")
    nc.sync.dma_start(cwbt[:], cwb[:])

    def wblk(i, rows=P):
        return cwbt[0:rows, i * P:(i + 1) * P]

    QUw = cwt[:, 0:P]
    QDw = cwt[:, P:2 * P]
    T3 = wblk(0)
    L32 = wblk(1)
    hT3 = wblk(2, 2)
    VsP, VsM, VdH, VdC = (wblk(i) for i in range(3, 7))
    hBP, hBM, hDH, hDC = (wblk(i, 2) for i in range(7, 11))

    sp_t = [None] * NB      # [P, WP] f32r padded s-hi
    sl_t = [None] * NB      # [P, WP] f32r padded s-lo
    q_t = [None] * NB       # [P, WP] f32 padded q
    hs_t = [None] * NB      # [2, WP] f32r s-hi halo rows (-1, 128)
    col_t = [None] * NB     # [P, W] bf16 horizontal 3-sum of tkm
    hc_t = [None] * NB      # [2, W] bf16 col halo rows
    lo_t = [None] * NB
    pi_t = [None] * NB
    arct_hold = [None]

    for it in range(NB + 3):
        # ------------- stage 0: load + round s (hi, lo) to f32r -------------
        b = it
        if b < NB:
            sp = spp.tile([P, WP], BF16, tag="sp")
            sp_t[b] = sp
            gp.memset(sp[:, 0:1], 0.0)
            gp.memset(sp[:, W + 1:W + 2], 0.0)
            nc.sync.dma_start(sp[:, 1:W + 1], sh_in[b * P:(b + 1) * P, :])
            sl = spp.tile([P, WP], BF16, tag="sl")
            sl_t[b] = sl
            gp.memset(sl[:, 0:1], 0.0)
            gp.memset(sl[:, W + 1:W + 2], 0.0)
            nc.sync.dma_start(sl[:, 1:W + 1], sl_in[b * P:(b + 1) * P, :])
            if b == 0:
                h0 = hpool.tile([2, WP], BF16, tag="hs")
                hs_t[0] = h0
                gp.memset(h0[0:1, :], 0.0)
            if b < NB - 1:
                hn = hpool.tile([2, WP], BF16, tag="hs")
                hs_t[b + 1] = hn
                if b + 1 == NB - 1:
                    gp.memset(hn[0:2, :], 0.0)
                nc.sync.dma_start(hn[0:1, :], sp[P - 1:P, :])
            if b >= 1:
                nc.sync.dma_start(hs_t[b - 1][1:2, :], sp[0:1, :])

        # ------------- stage 1: gx, gy, q, orientation -------------
        j = it - 1
        if 0 <= j < NB:
            sp = sp_t[j]
            sl = sl_t[j]
            hsp = hs_t[j]
            # 3*gx = VsP @ s[:, c+1] + VsM @ s[:, c-1]  (hi+lo, + halos)
            ps_gx = psA.tile([P, W], F32, tag="psA")
            for h in (0, HALF):
                o = ps_gx[:, h:h + HALF]
                te_.matmul(o, VsP, sp[:, h + 2:h + 2 + HALF], start=True,
                           stop=False)
                te_.matmul(o, VsP, sl[:, h + 2:h + 2 + HALF], start=False,
                           stop=False)
                te_.matmul(o, VsM, sp[:, h:h + HALF], start=False, stop=False)
                te_.matmul(o, VsM, sl[:, h:h + HALF], start=False, stop=False)
                te_.matmul(o, hBP, hsp[:, h + 2:h + 2 + HALF], start=False,
                           stop=False)
                te_.matmul(o, hBM, hsp[:, h:h + HALF], start=False, stop=True)
            gxo = outp.tile([P, W], F32, tag="gxo")
            sc.activation(gxo[:], ps_gx[:], AF.Copy, bias=GXEPS, scale=INV3)
            nc.sync.dma_start(o_gx[j * P:(j + 1) * P, :], gxo[:])
            gx2 = g2p.tile([P, W], F32, tag="gx2")
            sc.activation(gx2[:], ps_gx[:], AF.Square, scale=INV3)

            ps_gy = psA.tile([P, W], F32, tag="psA")
            for h in (0, HALF):
                o = ps_gy[:, h:h + HALF]
                te_.matmul(o, VdH, sp[:, h:h + HALF], start=True, stop=False)
                te_.matmul(o, VdH, sl[:, h:h + HALF], start=False, stop=False)
                te_.matmul(o, VdC, sp[:, h + 1:h + 1 + HALF], start=False,
                           stop=False)
                te_.matmul(o, VdC, sl[:, h + 1:h + 1 + HALF], start=False,
                           stop=False)
                te_.matmul(o, VdH, sp[:, h + 2:h + 2 + HALF], start=False,
                           stop=False)
                te_.matmul(o, VdH, sl[:, h + 2:h + 2 + HALF], start=False,
                           stop=False)
                te_.matmul(o, hDH, hsp[:, h:h + HALF], start=False,
                           stop=False)
                te_.matmul(o, hDC, hsp[:, h + 1:h + 1 + HALF], start=False,
                           stop=False)
                te_.matmul(o, hDH, hsp[:, h + 2:h + 2 + HALF], start=False,
                           stop=True)
            gyo = outp.tile([P, W], F32, tag="gyo")
            sc.activation(gyo[:], ps_gy[:], AF.Copy, scale=INV3)
            nc.sync.dma_start(o_gy[j * P:(j + 1) * P, :], gyo[:])
            gy2 = g2p.tile([P, W], F32, tag="gy2")
            sc.activation(gy2[:], ps_gy[:], AF.Square, scale=INV3)

            # q = gx2 + gy2 (exact f32), padded for the column shifts
            q = qpool.tile([P, WP], F32, tag="q")
            q_t[j] = q
            gp.memset(q[:, 0:1], 0.0)
            gp.memset(q[:, W + 1:W + 2], 0.0)
            v.tensor_tensor(q[:, 1:W + 1], gx2[:], gy2[:], ALU.add)

            # orientation: r = gy * rec(gx); o1 = round(arctan(r)*8/pi + 4)
            rec = sb1.tile([P, W], F32, tag="rec")
            v.reciprocal_approx_fast(rec[:], gxo[:])
            rmul = sb1.tile([P, W], F32, tag="rmul")
            v.tensor_tensor(rmul[:], gyo[:], rec[:], ALU.mult)
            arct = sb1.tile([P, W], F32, tag="arct")
            sc.activation(arct[:], rmul[:], AF.Arctan)
            arct_hold[0] = arct
            o1i = sb1.tile([P, W], I32, tag="o1i")
            v.tensor_scalar(o1i[:], arct[:], K8PI, 4.0, ALU.mult, ALU.add)
            oro = outp.tile([P, W], F32, tag="oro")
            sc.activation(oro[:], o1i[:], AF.Copy, scale=45.0)
            nc.sync.dma_start(o_or[j * P:(j + 1) * P, :], oro[:])
            pi_ = pip.tile([P, W], I32, tag="pi")
            pi_t[j] = pi_
            v.tensor_scalar(pi_[:], o1i[:], 3, None, ALU.bitwise_and)

        # ---------------- stage 2: NMS + thresholds ----------------
        k = it - 2
        if 0 <= k < NB:
            q = q_t[k]
            qc = q[:, 1:W + 1]
            # row shifts on PE in plain fp32 (exact); boundary rows patched
            # by 1-row DMAs from the neighbour q tiles
            ps_up = psU.tile([P, W], F32, tag="psU")
            ps_dn = psD.tile([P, W], F32, tag="psD")
            for h in (0, HALF):
                te_.matmul(ps_up[:, h:h + HALF], QUw,
                           qc[:, h:h + HALF], start=True, stop=True)
                te_.matmul(ps_dn[:, h:h + HALF], QDw,
                           qc[:, h:h + HALF], start=True, stop=True)
            quf = updn.tile([P, WP], F32, tag="quf")
            sc.activation(quf[:, 1:W + 1], ps_up[:], AF.Copy)
            gp.memset(quf[:, 0:1], 0.0)
            gp.memset(quf[:, W + 1:W + 2], 0.0)
            qdf = updn.tile([P, WP], F32, tag="qdf")
            sc.activation(qdf[:, 1:W + 1], ps_dn[:], AF.Copy)
            gp.memset(qdf[:, 0:1], 0.0)
            gp.memset(qdf[:, W + 1:W + 2], 0.0)
            if k >= 1:
                nc.sync.dma_start(quf[0:1, 1:W + 1], q_t[k - 1][127:128, 1:W + 1])
            if k < NB - 1:
                nc.sync.dma_start(qdf[127:128, 1:W + 1], q_t[k + 1][0:1, 1:W + 1])

            ms = msk.tile([P, 2 * W], U8, tag="m")
            pi_ = pi_t[k]
            v.tensor_scalar(ms[:, 0:W], pi_[:], 2, None, ALU.is_ge)
            v.tensor_scalar(ms[:, W:2 * W], pi_[:], 3, None, ALU.is_equal)
            M = sb1.tile([P, W], F32, tag="M")
            v.tensor_tensor(M[:], q[:, 0:W], q[:, 2:W + 2], ALU.max)
            M1 = sb1.tile([P, W], F32, tag="M1")
            v.tensor_tensor(M1[:], quf[:, 2:W + 2], qdf[:, 0:W], ALU.max)
            M2 = sb1.tile([P, W], F32, tag="M2")
            v.tensor_tensor(M2[:], quf[:, 1:W + 1], qdf[:, 1:W + 1],
                            ALU.max)
            M3 = sb1.tile([P, W], F32, tag="M3")
            v.tensor_tensor(M3[:], quf[:, 0:W], qdf[:, 2:W + 2], ALU.max)
            v.copy_predicated(M[:], pi_[:], M1[:])
            v.copy_predicated(M[:], ms[:, 0:W], M2[:])
            v.copy_predicated(M[:], ms[:, W:2 * W], M3[:])

            km = thp.tile([P, W], BF16, tag="km")
            v.tensor_tensor(km[:], qc, M[:], ALU.is_gt)
            lo1 = thp.tile([P, W], BF16, tag="lo1")
            v.tensor_scalar(lo1[:], qc, 0.25, None, ALU.is_gt)
            hi1 = thp.tile([P, W], BF16, tag="hi1")
            v.tensor_scalar(hi1[:], qc, 1.0, None, ALU.is_gt)
            thr = thp.tile([P, W], BF16, tag="thr")
            v.tensor_tensor(thr[:], lo1[:], hi1[:], ALU.add)
            btp = thp.tile([P, WP], BF16, tag="btp")
            gp.memset(btp[:, 0:1], 0.0)
            gp.memset(btp[:, W + 1:W + 2], 0.0)
            v.tensor_tensor(btp[:, 1:W + 1], thr[:], km[:], ALU.mult)
            lo = thp2.tile([P, W], BF16, tag="lo")
            lo_t[k] = lo
            v.tensor_tensor(lo[:], lo1[:], km[:], ALU.mult)
            ca = thp.tile([P, W], BF16, tag="ca")
            v.tensor_tensor(ca[:], btp[:, 0:W], btp[:, 2:W + 2], ALU.add)
            col = thp2.tile([P, W], BF16, tag="col")
            col_t[k] = col
            v.tensor_tensor(col[:], ca[:], btp[:, 1:W + 1], ALU.add)
            if k == 0:
                c0 = hpool.tile([2, W], BF16, tag="hc")
                hc_t[0] = c0
                gp.memset(c0[0:1, :], 0.0)
            if k < NB - 1:
                cn = hpool.tile([2, W], BF16, tag="hc")
                hc_t[k + 1] = cn
                if k + 1 == NB - 1:
                    gp.memset(cn[0:2, :], 0.0)
                nc.sync.dma_start(cn[0:1, :], col[P - 1:P, :])
            if k >= 1:
                nc.sync.dma_start(hc_t[k - 1][1:2, :], col[0:1, :])

        # ---------------- stage 3: hysteresis ----------------
        l = it - 3
        if 0 <= l < NB:
            ps_S = psA.tile([P, W], F32, tag="psA")
            for h in (0, HALF):
                o = ps_S[:, h:h + HALF]
                te_.matmul(o, T3, col_t[l][:, h:h + HALF], start=True,
                           stop=False)
                te_.matmul(o, hT3, hc_t[l][:, h:h + HALF], start=False,
                           stop=False)
                te_.matmul(o, L32, lo_t[l][:, h:h + HALF], start=False,
                           stop=True)
            fin = outp.tile([P, W], F32, tag="fin")
            v.tensor_scalar(fin[:], ps_S[:], 33.5, None, ALU.is_ge)
            nc.sync.dma_start(o_te[l * P:(l + 1) * P, :], fin[:])

    # ---------------- sqrt phase (separate ACT table set) ----------------
    z = cpool.tile([P, 1], F32, tag="z")
    v.tensor_scalar_mul(z[:], arct_hold[0][:, 0:1], 0.0)
    for j in range(NB):
        gm = outp.tile([P, W], F32, tag="gm")
        sc.activation(gm[:], q_t[j][:, 1:W + 1], AF.Sqrt, bias=z[:])
        nc.sync.dma_start(o_gm[j * P:(j + 1) * P, :], gm[:])

    ctx.close()


def _build():
    nc = bacc.Bacc()
    sh_in = nc.declare_dram_parameter("sh_in", [H, W], BF16, isOutput=False)
    sl_in = nc.declare_dram_parameter("sl_in", [H, W], BF16, isOutput=False)
    cw = nc.declare_dram_parameter("cw", [P, 2 * P], F32, isOutput=False)
    cwb = nc.declare_dram_parameter("cwb", [P, 11 * P], BF16, isOutput=False)
    outs = {nm: nc.declare_dram_parameter(nm, [H, W], F32, isOutput=True)
            for nm in ("o_gx", "o_gy", "o_gm", "o_or", "o_te")}
    with tile.TileContext(nc) as tc:
        _emit(nc, tc, sh_in, sl_in, cw, cwb, outs["o_gx"], outs["o_gy"],
              outs["o_gm"], outs["o_or"], outs["o_te"])
    nc.finalize()
    return nc


_NC_CACHE = None


def _get_nc():
    global _NC_CACHE
    if _NC_CACHE is None:
        _NC_CACHE = _build()
    return _NC_CACHE


LAST_RESULTS = None


def kernel(img: np.ndarray):
    global LAST_RESULTS
    img = np.asarray(img, np.float32)
    B = img.shape[0]
    s = img.sum(axis=1, dtype=np.float32)  # (B, H, W)
    sh = s.astype(ml_dtypes.bfloat16)
    slo = (s - sh.astype(np.float32)).astype(ml_dtypes.bfloat16)
    cw = _const_weights()
    cwb = _const_weights_bf16()
    nc = _get_nc()
    in_maps = [{"sh_in": np.ascontiguousarray(sh[i]),
                "sl_in": np.ascontiguousarray(slo[i]),
                "cw": cw, "cwb": cwb} for i in range(B)]
    trace = bool(int(os.environ.get("KTRACE", "0")))
    out = run_bass_kernel_spmd(nc, in_maps, list(range(B)), trace=trace)
    LAST_RESULTS = out
    res = out.results
    gx = np.stack([res[i]["o_gx"] for i in range(B)])[:, None]
    gy = np.stack([res[i]["o_gy"] for i in range(B)])[:, None]
    gm = np.stack([res[i]["o_gm"] for i in range(B)])[:, None]
    orient = np.stack([res[i]["o_or"] for i in range(B)])[:, None]
    edges = np.stack([res[i]["o_te"] for i in range(B)])[:, None]
    return (gx, gy, gm, orient, edges)
